# revision 22
# baseline (speedup 1.0000x reference)
"""Trainium2 Bass kernel for nn_Attention_90486370992549.

Learned-sigmoid-mask multi-head attention:
  qkv = x @ W_qkv.T + b_qkv
  attn = softmax((q k^T / sqrt(D)) * sigmoid(att_mask))
  out  = (attn @ v) @ W_proj.T + b_proj

Sharding: data-parallel over batch across 8 NeuronCores (16 batches/core).
All matmuls run in bf16 (full 1 cycle/row PE rate at any moving size, FWL
weight loads); accumulation is fp32 in PSUM. x is transposed host-side, so
x^T tiles stream straight from DRAM with no on-chip transpose.

The pass is software-pipelined at attention-pair granularity. For chunk ck
(2 batches, 12 (batch, head-pair) pairs), each pair emits:
  - S^T matmuls of pair p (K=64 even/odd heads in PE row groups 0/64)
  - PV matmuls of pair p-2 (so mask-mult/exp latency never stalls the PE),
    with a ones-column in V producing the softmax denominator as PSUM row 64
  - one slice of front-work for chunk ck+1 (x^T DMA / V / qk^T matmuls)
  - one half of a proj tile of chunk ck-1
Softmax normalization: ACT reciprocal of the denominator row, GpSimd
partition-broadcast of 1/den (engines cannot broadcast across partitions),
then the multiply is fused into the PSUM->SBUF copy of the attention
output - even heads on DVE, odd heads via ACT copy + GpSimd multiply so no
single engine becomes the bottleneck.
"""

import numpy as np

B, N, C, H, D = 128, 196, 768, 12, 64
SCALE = D ** -0.5
NCORES = 8
BPC = B // NCORES              # batches per core
BPCHUNK = 2                    # batches per chunk
NCHUNK = BPC // BPCHUNK        # 8 chunks
T = BPCHUNK * N                # 392 tokens per chunk
TOK_TILES = [(0, 128), (128, 128), (256, 128), (384, 8)]
MC = [(0, 98), (98, 98)]       # m-chunks within one batch (even split so
                               # both halves write/read the same lane count)
NPAIR = 12                     # (batch, head-pair) pairs per chunk
PVLAG = 3                      # PV of pair p emitted at pair p+PVLAG

_CACHE = {}


def _build(repeat=1, loop=0, stage=3):
    from contextlib import ExitStack, nullcontext

    import concourse.bacc as bacc
    import concourse.bass as bass
    import concourse.mybir as mybir
    from concourse.tile import TileContext

    f32 = mybir.dt.float32
    bf16 = mybir.dt.bfloat16
    AF = mybir.ActivationFunctionType
    OP = mybir.AluOpType

    nc = bacc.Bacc("TRN2", target_bir_lowering=False, debug=False,
                   num_devices=NCORES)
    xT = nc.dram_tensor("xT", [C, BPC * N], bf16, kind="ExternalInput")
    wqkT = nc.dram_tensor("wqkT", [C, 2 * C], bf16, kind="ExternalInput")
    wvT = nc.dram_tensor("wvT", [C, C], bf16, kind="ExternalInput")
    wpT = nc.dram_tensor("wpT", [C, C], bf16, kind="ExternalInput")
    bqk = nc.dram_tensor("bqk", [128, 12], f32, kind="ExternalInput")
    bv = nc.dram_tensor("bv", [1, C], f32, kind="ExternalInput")
    bp = nc.dram_tensor("bp", [1, C], f32, kind="ExternalInput")
    maskA = nc.dram_tensor("maskA", [98, H, N], bf16, kind="ExternalInput")
    maskB = nc.dram_tensor("maskB", [98, H, N], bf16, kind="ExternalInput")
    y = nc.dram_tensor("y", [BPC * N, C], f32, kind="ExternalOutput")
    xT_r = xT.rearrange("(j p) t -> p j t", p=128)

    with TileContext(nc) as tc, ExitStack() as ctx:
        singles = ctx.enter_context(tc.tile_pool(name="singles", bufs=1))
        xT_p = ctx.enter_context(tc.tile_pool(name="xT", bufs=2))
        qkT_p = ctx.enter_context(tc.tile_pool(name="qkT", bufs=2))
        v_p = ctx.enter_context(tc.tile_pool(name="v", bufs=8))
        ot_p = ctx.enter_context(tc.tile_pool(name="ot", bufs=2))
        p_p = ctx.enter_context(tc.tile_pool(name="p", bufs=6))
        y_p = ctx.enter_context(tc.tile_pool(name="y", bufs=2))
        rc_p = ctx.enter_context(tc.tile_pool(name="rc", bufs=3))
        bc_p = ctx.enter_context(tc.tile_pool(name="bc", bufs=3))
        or_p = ctx.enter_context(tc.tile_pool(name="or", bufs=3))
        dram_p = ctx.enter_context(tc.tile_pool(name="dram", bufs=3,
                                                space="DRAM"))
        # PSUM: 8 banks total. pq/pv share 2 (disjoint in time), s_t 2,
        # po 2, ph 2.
        ps_ms = ctx.enter_context(tc.tile_pool(name="psms", bufs=2,
                                               space="PSUM"))
        ps_st = ctx.enter_context(tc.tile_pool(name="psst", bufs=2,
                                               space="PSUM"))
        ps_o = ctx.enter_context(tc.tile_pool(name="pso", bufs=2,
                                              space="PSUM"))
        ps_ph = ctx.enter_context(tc.tile_pool(name="psph", bufs=2,
                                               space="PSUM"))

        # --- resident weights / constants ---
        # All weight loads go on the scalar HWDGE ring, in need-order (qk
        # weights first, proj weights last), so the per-chunk xT/y DMAs on
        # the sync ring never queue behind them.
        bqk_sb = singles.tile([128, 12], f32)
        nc.scalar.dma_start(bqk_sb[:], bqk[:])
        wqk_sb = singles.tile([128, 6, 2 * C], bf16)
        _wqk_r = wqkT.rearrange("(ko p) n -> p ko n", p=128)
        _splits = [0, 128, 384, 768, 1152, 1536]
        for _a in range(len(_splits) - 1):
            nc.scalar.dma_start(wqk_sb[:, :, _splits[_a]:_splits[_a + 1]],
                                _wqk_r[:, :, _splits[_a]:_splits[_a + 1]])
        mA_sb = singles.tile([98, H, N], bf16)
        nc.scalar.dma_start(mA_sb[:], maskA[:])
        mB_sb = singles.tile([98, H, N], bf16)
        nc.scalar.dma_start(mB_sb[:], maskB[:])
        wv_sb = singles.tile([128, 6, C], bf16)
        nc.scalar.dma_start(wv_sb[:],
                            wvT.rearrange("(ko p) n -> p ko n", p=128))
        bv_sb = singles.tile([128, C], f32)
        bv_ap = bv.ap()
        nc.scalar.dma_start(bv_sb[:], bass.AP(
            tensor=bv_ap.tensor, offset=bv_ap.offset,
            ap=[[0, 128], bv_ap.ap[1]]))
        wp_sb = singles.tile([128, 6, C], bf16)
        nc.scalar.dma_start(wp_sb[:],
                            wpT.rearrange("(ko p) n -> p ko n", p=128))
        bp_sb = singles.tile([128, C], f32)
        bp_ap = bp.ap()
        nc.scalar.dma_start(bp_sb[:], bass.AP(
            tensor=bp_ap.tensor, offset=bp_ap.offset,
            ap=[[0, 128], bp_ap.ap[1]]))

        # ---------- front work: prepare chunk ck's xT / V / qkT ----------
        def front_xt(st, ck):
            t = xT_p.tile([128, 6, T], bf16, tag="xT")
            nc.sync.dma_start(t[:], xT_r[:, :, ck * T:(ck + 1) * T])
            st["xT"] = t
            # even/odd heads in separate partition-base-0 tiles: bf16 K=64
            # matmuls with alternating PE row offsets lock up the device, so
            # every S matmul must read its operands at partition base 0
            st["qkTe"] = qkT_p.tile([64, 12, T], bf16, tag="qkTe",
                                    name="qkTe")
            st["qkTo"] = qkT_p.tile([64, 12, T], bf16, tag="qkTo",
                                    name="qkTo")
            st["vts"] = []

        def front_v_slice(st, s):
            b, mi = divmod(s, 2)
            moff, mrows = MC[mi]
            soff = b * N + moff
            vt = v_p.tile([128, H, D + 1], bf16, tag="v")
            pv = [ps_ms.tile([128, 384], f32, tag="ms", name="pv")[:mrows]
                  for _ in range(2)]
            for j in range(6):
                lhs = st["xT"][:, j, soff:soff + mrows]
                for half in range(2):
                    nc.tensor.matmul(
                        pv[half], lhs,
                        wv_sb[:, j, half * 384:(half + 1) * 384],
                        start=(j == 0), stop=(j == 5))
            for half in range(2):
                nc.vector.tensor_tensor(
                    vt[:mrows, half * 6:(half + 1) * 6, :D],
                    pv[half].rearrange("p (h d) -> p h d", d=D),
                    bv_sb[:mrows, half * 384:(half + 1) * 384]
                    .rearrange("p (h d) -> p h d", d=D),
                    OP.add)
            nc.gpsimd.memset(vt[:mrows, :, D:D + 1], 1.0)
            st["vts"].append(vt)

        def front_qk_tile(st, i):
            pq = ps_ms.tile([128, T], f32, tag="ms", name="pq")
            for j in range(6):
                nc.tensor.matmul(
                    pq[:], wqk_sb[:, j, i * 128:(i + 1) * 128],
                    st["xT"][:, j, :], start=(j == 0), stop=(j == 5))
            nc.scalar.activation(st["qkTe"][:, i, :], pq[0:64, :],
                                 AF.Identity, bias=bqk_sb[0:64, i:i + 1])
            nc.scalar.activation(st["qkTo"][:, i, :], pq[64:128, :],
                                 AF.Identity, bias=bqk_sb[64:128, i:i + 1])

        def front_piece(st, ck, piece):
            """piece 0: xT DMA (latency hidden behind pairs 0-1);
            pieces 2-5: one V slice each; pieces 6-11: qk tiles 2i, 2i+1."""
            if piece == 0:
                front_xt(st, ck)
            elif 2 <= piece < 6:
                front_v_slice(st, piece - 2)
            elif 6 <= piece < 12:
                front_qk_tile(st, 2 * (piece - 6))
                front_qk_tile(st, 2 * (piece - 6) + 1)

        def front_all(st, ck):
            # prologue order: xT DMA, then qk (pairs need it immediately),
            # then V (first needed at pair PVLAG)
            for piece in [0, 6, 7, 8, 9, 10, 11, 2, 3, 4, 5]:
                front_piece(st, ck, piece)

        # ---------- back work: proj of a finished chunk ----------
        phs = {}

        def emit_proj_half(ot, ck, ti, half):
            off, rows = TOK_TILES[ti]
            ph = ps_ph.tile([128, 384], f32, tag="ph", name="ph")[:rows]
            for j in range(6):
                nc.tensor.matmul(
                    ph, ot[:, j, off:off + rows],
                    wp_sb[:, j, half * 384:(half + 1) * 384],
                    start=(j == 0), stop=(j == 5))
            if half == 0:
                phs[ti % 2] = (ph, y_p.tile([128, C], f32, tag="y", name="ysb"))
            else:
                phs[ti % 2] = (phs[ti % 2][0], phs[ti % 2][1], ph)
                ph0, y_sb, ph1 = phs[ti % 2]
                for h, p in ((0, ph0), (1, ph1)):
                    nc.vector.tensor_tensor(
                        y_sb[:rows, h * 384:(h + 1) * 384], p,
                        bp_sb[:rows, h * 384:(h + 1) * 384], OP.add)
                nc.scalar.dma_start(
                    y[ck * T + off: ck * T + off + rows, :], y_sb[:rows])

        def emit_proj(ot, ck):
            for ti in range(len(TOK_TILES)):
                for half in range(2):
                    emit_proj_half(ot, ck, ti, half)

        # ---------- attention ----------
        def emit_s(st, p):
            b, j = divmod(p, 6)
            sts = []
            for mi, (moff, mrows) in enumerate(MC):
                s_t = ps_st.tile([98, 2, N], f32, tag="st")
                for hp in range(2):
                    qs = st["qkTe"] if hp == 0 else st["qkTo"]
                    k_ap = qs[:, 6 + j, b * N + moff: b * N + moff + mrows]
                    q_ap = qs[:, j, b * N: (b + 1) * N]
                    nc.tensor.matmul(
                        s_t[:mrows, hp, :], k_ap, q_ap,
                        start=True, stop=True)
                sts.append(s_t)
            return sts

        def emit_p(st, p, sts):
            b, j = divmod(p, 6)
            pt = p_p.tile([128, 4, N], bf16, tag="p")
            for mi in range(2):
                m_sb = (mA_sb if mi == 0 else mB_sb)
                nc.vector.tensor_tensor(
                    pt[:98, 2 * mi:2 * mi + 2, :], sts[mi][:],
                    m_sb[:, 2 * j:2 * j + 2, :], OP.mult)
            nc.scalar.activation(pt[:98], pt[:98], AF.Exp)
            return pt

        def emit_pv(st, ot, pend):
            b, j, pt = pend
            po = ps_o.tile([D + 1, 2, N], f32, tag="o")
            for hp in range(2):
                for mi, (moff, mrows) in enumerate(MC):
                    nc.tensor.matmul(
                        po[:, hp, :],
                        st["vts"][b * 2 + mi][:mrows, 2 * j + hp, :],
                        pt[:mrows, 2 * mi + hp, :],
                        start=(mi == 0), stop=(mi == 1))
            # unload PSUM immediately (frees the po slot): reciprocal of the
            # denominator row on DVE (no ACT table holds exp+reciprocal, so
            # ACT reciprocal would thrash 1.3us table reloads), raw attention
            # rows to SBUF on ACT. Normalization happens per batch in
            # emit_norm_batch.
            if "rt" not in st or st["rt_b"] != b:
                st["rt"] = rc_p.tile([1, 6, 2, N], f32, tag="rc", name="rt")
                st["orb"] = or_p.tile([64, 6, 2, N], bf16, tag="or",
                                      name="orb")
                st["rt_b"] = b
            nc.vector.reciprocal(st["rt"][:, j, :, :], po[D:D + 1, :, :])
            nc.scalar.activation(st["orb"][:, j, :, :], po[:D, :, :],
                                 AF.Copy)
            return (b, st["rt"], st["orb"])

        def emit_norm_batch(ot, b, rt, orb):
            """Batched softmax normalization: one DRAM round-trip broadcasts
            1/den for all 6 head-pairs of a batch (engines cannot
            partition-broadcast), then two wide GpSimd multiplies write the
            normalized attention output (partition-base shift for odd heads
            happens on GpSimd)."""
            scr = dram_p.tile([1, 6, 2, N], f32, tag="scr", name="scr")
            nc.sync.dma_start(scr[:], rt[:])
            bcast = bc_p.tile([64, 6, 2, N], f32, tag="bc")
            scr_ap = scr[:]
            nc.sync.dma_start(
                bcast[:],
                bass.AP(tensor=scr_ap.tensor, offset=scr_ap.offset,
                        ap=[[0, 64], [2 * N, 6], [N, 2], [1, N]]))
            for hp in range(2):
                nc.gpsimd.tensor_tensor(
                    ot[hp * 64:(hp + 1) * 64, :, b * N:(b + 1) * N],
                    orb[:, :, hp, :], bcast[:, :, hp, :], OP.mult)

        loop_cm = tc.For_i(0, loop, 1) if loop else nullcontext()
        with loop_cm:
          for _rep in range(repeat):
            # chunks[ck] = {"st": front state, "ot": attention output tile}
            chunks = {0: {"st": {}, "ot": None}}
            front_all(chunks[0]["st"], 0)
            if stage == 1:
                nc.sync.dma_start(
                    y[0:128, 0:196],
                    chunks[0]["st"]["qkTe"][:, 0, :].bitcast(f32))
                continue
            pends = []   # (ck, (b, j, pt)) awaiting PV
            nchunk = 1 if stage == 2 else NCHUNK
            for ck in range(nchunk):
                rec = chunks[ck]
                rec["ot"] = ot_p.tile([128, 6, T], bf16, tag="ot", name="ott")
                if ck + 1 < nchunk:
                    chunks[ck + 1] = {"st": {}, "ot": None}
                for p in range(NPAIR):
                    sts = emit_s(rec["st"], p)
                    if len(pends) >= PVLAG:
                        cko, pend = pends.pop(0)
                        done = emit_pv(chunks[cko]["st"], chunks[cko]["ot"],
                                       pend)
                        if pend[1] == 5:
                            emit_norm_batch(chunks[cko]["ot"], *done)
                    if ck + 1 < nchunk:
                        front_piece(chunks[ck + 1]["st"], ck + 1, p)
                    if ck >= 1 and 4 <= p < 12:
                        ti, half = divmod(p - 4, 2)
                        emit_proj_half(chunks[ck - 1]["ot"], ck - 1, ti, half)
                    pt = emit_p(rec["st"], p, sts)
                    pends.append((ck, (p // 6, p % 6, pt)))
                if ck - 2 in chunks:
                    del chunks[ck - 2]
            # epilogue: flush remaining PVs, then last chunk's proj
            while pends:
                cko, pend = pends.pop(0)
                done = emit_pv(chunks[cko]["st"], chunks[cko]["ot"], pend)
                if pend[1] == 5:
                    emit_norm_batch(chunks[cko]["ot"], *done)
            if stage == 2:
                nc.sync.dma_start(
                    y[0:128, 0:196],
                    chunks[0]["ot"][:, 0, :].bitcast(f32))
                continue
            emit_proj(chunks[NCHUNK - 1]["ot"], NCHUNK - 1)

    nc.compile()
    return nc


def _get_nc(repeat=1, loop=0, stage=3):
    key = ("nc", repeat, loop, stage)
    if key not in _CACHE:
        _CACHE[key] = _build(repeat, loop, stage)
    return _CACHE[key]


def _prep_shared(W_qkv, b_qkv, att_mask, W_proj, b_proj):
    import ml_dtypes
    bf16 = ml_dtypes.bfloat16
    W_qkv = np.asarray(W_qkv, np.float32)
    W_proj = np.asarray(W_proj, np.float32)
    b_qkv = np.asarray(b_qkv, np.float32)
    b_proj = np.asarray(b_proj, np.float32)
    att_mask = np.asarray(att_mask, np.float32)
    sig = SCALE / (1.0 + np.exp(-att_mask))          # [H, n, m]
    maskT = np.ascontiguousarray(sig.transpose(0, 2, 1))  # [H, m, n]
    return {
        "wqkT": np.ascontiguousarray(W_qkv[:2 * C].T).astype(bf16),
        "wvT": np.ascontiguousarray(W_qkv[2 * C:].T).astype(bf16),
        "wpT": np.ascontiguousarray(W_proj.T).astype(bf16),
        "bqk": np.ascontiguousarray(b_qkv[:2 * C].reshape(12, 128).T),
        "bv": np.ascontiguousarray(b_qkv[2 * C:].reshape(1, C)),
        "bp": np.ascontiguousarray(b_proj.reshape(1, C)),
        "maskA": np.ascontiguousarray(
            maskT[:, :98, :].transpose(1, 0, 2)).astype(bf16),
        "maskB": np.ascontiguousarray(
            maskT[:, 98:, :].transpose(1, 0, 2)).astype(bf16),
    }


def _make_in_maps(x, W_qkv, b_qkv, att_mask, W_proj, b_proj):
    import ml_dtypes
    bf16 = ml_dtypes.bfloat16
    x = np.asarray(x, np.float32).astype(bf16)
    shared = _prep_shared(W_qkv, b_qkv, att_mask, W_proj, b_proj)
    in_maps = []
    for c in range(NCORES):
        m = dict(shared)
        m["xT"] = np.ascontiguousarray(
            x[c * BPC:(c + 1) * BPC].reshape(BPC * N, C).T)
        in_maps.append(m)
    return in_maps


def kernel(x, W_qkv, b_qkv, att_mask, W_proj, b_proj):
    from concourse.bass_utils import run_bass_kernel_spmd

    nc = _get_nc()
    in_maps = _make_in_maps(x, W_qkv, b_qkv, att_mask, W_proj, b_proj)
    res = run_bass_kernel_spmd(nc, in_maps, core_ids=list(range(NCORES)))
    out = np.stack([res.results[c]["y"].reshape(BPC, N, C)
                    for c in range(NCORES)])
    return out.reshape(B, N, C).astype(np.float32)


# revision 23
# speedup vs baseline: 1.2941x; 1.2941x over previous
"""Trainium2 Bass kernel for nn_Attention_90486370992549.

Learned-sigmoid-mask multi-head attention:
  qkv = x @ W_qkv.T + b_qkv
  attn = softmax((q k^T / sqrt(D)) * sigmoid(att_mask))
  out  = (attn @ v) @ W_proj.T + b_proj

Sharding: data-parallel over batch across 8 NeuronCores (16 batches/core).
All matmuls run in bf16 (full 1 cycle/row PE rate at any moving size, FWL
weight loads); accumulation is fp32 in PSUM. x is transposed host-side, so
x^T tiles stream straight from DRAM with no on-chip transpose.

The pass is software-pipelined at attention-pair granularity. For chunk ck
(2 batches, 12 (batch, head-pair) pairs), each pair emits:
  - S^T matmuls of pair p (K=64 even/odd heads in PE row groups 0/64)
  - PV matmuls of pair p-2 (so mask-mult/exp latency never stalls the PE),
    with a ones-column in V producing the softmax denominator as PSUM row 64
  - one slice of front-work for chunk ck+1 (x^T DMA / V / qk^T matmuls)
  - one half of a proj tile of chunk ck-1
Softmax normalization: ACT reciprocal of the denominator row, GpSimd
partition-broadcast of 1/den (engines cannot broadcast across partitions),
then the multiply is fused into the PSUM->SBUF copy of the attention
output - even heads on DVE, odd heads via ACT copy + GpSimd multiply so no
single engine becomes the bottleneck.
"""

import numpy as np

B, N, C, H, D = 128, 196, 768, 12, 64
SCALE = D ** -0.5
NCORES = 8
BPC = B // NCORES              # batches per core
BPCHUNK = 2                    # batches per chunk
NCHUNK = BPC // BPCHUNK        # 8 chunks
T = BPCHUNK * N                # 392 tokens per chunk
TOK_TILES = [(0, 128), (128, 128), (256, 128), (384, 8)]
MC = [(0, 98), (98, 98)]       # m-chunks within one batch (even split so
                               # both halves write/read the same lane count)
NPAIR = 12                     # (batch, head-pair) pairs per chunk
PVLAG = 2                      # PV of pair p emitted at pair p+PVLAG

_CACHE = {}


def _build(repeat=1, loop=0, stage=3):
    from contextlib import ExitStack, nullcontext

    import concourse.bacc as bacc
    import concourse.bass as bass
    import concourse.mybir as mybir
    from concourse.tile import TileContext

    f32 = mybir.dt.float32
    bf16 = mybir.dt.bfloat16
    AF = mybir.ActivationFunctionType
    OP = mybir.AluOpType

    nc = bacc.Bacc("TRN2", target_bir_lowering=False, debug=False,
                   num_devices=NCORES)
    xT = nc.dram_tensor("xT", [C, BPC * N], bf16, kind="ExternalInput")
    wqkT = nc.dram_tensor("wqkT", [C, 2 * C], bf16, kind="ExternalInput")
    wvT = nc.dram_tensor("wvT", [C, C], bf16, kind="ExternalInput")
    wpT = nc.dram_tensor("wpT", [C, C], bf16, kind="ExternalInput")
    bqk = nc.dram_tensor("bqk", [128, 12], f32, kind="ExternalInput")
    bv = nc.dram_tensor("bv", [1, C], f32, kind="ExternalInput")
    bp = nc.dram_tensor("bp", [1, C], f32, kind="ExternalInput")
    maskA = nc.dram_tensor("maskA", [98, H, N], bf16, kind="ExternalInput")
    maskB = nc.dram_tensor("maskB", [98, H, N], bf16, kind="ExternalInput")
    y = nc.dram_tensor("y", [BPC * N, C], f32, kind="ExternalOutput")
    xT_r = xT.rearrange("(j p) t -> p j t", p=128)

    with TileContext(nc) as tc, ExitStack() as ctx:
        singles = ctx.enter_context(tc.tile_pool(name="singles", bufs=1))
        xT_p = ctx.enter_context(tc.tile_pool(name="xT", bufs=2))
        qkT_p = ctx.enter_context(tc.tile_pool(name="qkT", bufs=2))
        v_p = ctx.enter_context(tc.tile_pool(name="v", bufs=8))
        ot_p = ctx.enter_context(tc.tile_pool(name="ot", bufs=2))
        p_p = ctx.enter_context(tc.tile_pool(name="p", bufs=4))
        y_p = ctx.enter_context(tc.tile_pool(name="y", bufs=2))
        rc_p = ctx.enter_context(tc.tile_pool(name="rc", bufs=3))
        bc_p = ctx.enter_context(tc.tile_pool(name="bc", bufs=3))
        or_p = ctx.enter_context(tc.tile_pool(name="or", bufs=3))
        dram_p = ctx.enter_context(tc.tile_pool(name="dram", bufs=3,
                                                space="DRAM"))
        # PSUM: 8 banks total. pq/pv share 2 (disjoint in time), s_t 2,
        # po 2, ph 2.
        ps_ms = ctx.enter_context(tc.tile_pool(name="psms", bufs=2,
                                               space="PSUM"))
        ps_st = ctx.enter_context(tc.tile_pool(name="psst", bufs=2,
                                               space="PSUM"))
        ps_o = ctx.enter_context(tc.tile_pool(name="pso", bufs=2,
                                              space="PSUM"))
        ps_ph = ctx.enter_context(tc.tile_pool(name="psph", bufs=2,
                                               space="PSUM"))

        # --- resident weights / constants ---
        # All weight loads go on the scalar HWDGE ring, in need-order (qk
        # weights first, proj weights last), so the per-chunk xT/y DMAs on
        # the sync ring never queue behind them.
        bqk_sb = singles.tile([128, 12], f32)
        nc.scalar.dma_start(bqk_sb[:], bqk[:])
        wqk_sb = singles.tile([128, 6, 2 * C], bf16)
        _wqk_r = wqkT.rearrange("(ko p) n -> p ko n", p=128)
        _splits = [0, 128, 384, 768, 1152, 1536]
        for _a in range(len(_splits) - 1):
            nc.scalar.dma_start(wqk_sb[:, :, _splits[_a]:_splits[_a + 1]],
                                _wqk_r[:, :, _splits[_a]:_splits[_a + 1]])
        mA_sb = singles.tile([98, H, N], bf16)
        nc.scalar.dma_start(mA_sb[:], maskA[:])
        mB_sb = singles.tile([98, H, N], bf16)
        nc.scalar.dma_start(mB_sb[:], maskB[:])
        wv_sb = singles.tile([128, 6, C], bf16)
        nc.scalar.dma_start(wv_sb[:],
                            wvT.rearrange("(ko p) n -> p ko n", p=128))
        bv_sb = singles.tile([128, C], f32)
        bv_ap = bv.ap()
        nc.scalar.dma_start(bv_sb[:], bass.AP(
            tensor=bv_ap.tensor, offset=bv_ap.offset,
            ap=[[0, 128], bv_ap.ap[1]]))
        wp_sb = singles.tile([128, 6, C], bf16)
        nc.scalar.dma_start(wp_sb[:],
                            wpT.rearrange("(ko p) n -> p ko n", p=128))
        bp_sb = singles.tile([128, C], f32)
        bp_ap = bp.ap()
        nc.scalar.dma_start(bp_sb[:], bass.AP(
            tensor=bp_ap.tensor, offset=bp_ap.offset,
            ap=[[0, 128], bp_ap.ap[1]]))

        # ---------- front work: prepare chunk ck's xT / V / qkT ----------
        def front_xt(st, ck):
            t = xT_p.tile([128, 6, T], bf16, tag="xT")
            nc.sync.dma_start(t[:], xT_r[:, :, ck * T:(ck + 1) * T])
            st["xT"] = t
            # even/odd heads in separate partition-base-0 tiles: bf16 K=64
            # matmuls with alternating PE row offsets lock up the device, so
            # every S matmul must read its operands at partition base 0
            st["qkTe"] = qkT_p.tile([64, 12, T], bf16, tag="qkTe",
                                    name="qkTe")
            st["qkTo"] = qkT_p.tile([64, 12, T], bf16, tag="qkTo",
                                    name="qkTo")
            st["vts"] = []

        def front_v_slice(st, s):
            b, mi = divmod(s, 2)
            moff, mrows = MC[mi]
            soff = b * N + moff
            vt = v_p.tile([128, H, D + 1], bf16, tag="v")
            pv = [ps_ms.tile([128, 384], f32, tag="ms", name="pv")[:mrows]
                  for _ in range(2)]
            for j in range(6):
                lhs = st["xT"][:, j, soff:soff + mrows]
                for half in range(2):
                    nc.tensor.matmul(
                        pv[half], lhs,
                        wv_sb[:, j, half * 384:(half + 1) * 384],
                        start=(j == 0), stop=(j == 5))
            for half in range(2):
                nc.vector.tensor_tensor(
                    vt[:mrows, half * 6:(half + 1) * 6, :D],
                    pv[half].rearrange("p (h d) -> p h d", d=D),
                    bv_sb[:mrows, half * 384:(half + 1) * 384]
                    .rearrange("p (h d) -> p h d", d=D),
                    OP.add)
            nc.gpsimd.memset(vt[:mrows, :, D:D + 1], 1.0)
            st["vts"].append(vt)

        def front_qk_tile(st, i):
            pq = ps_ms.tile([128, T], f32, tag="ms", name="pq")
            for j in range(6):
                nc.tensor.matmul(
                    pq[:], wqk_sb[:, j, i * 128:(i + 1) * 128],
                    st["xT"][:, j, :], start=(j == 0), stop=(j == 5))
            nc.scalar.activation(st["qkTe"][:, i, :], pq[0:64, :],
                                 AF.Identity, bias=bqk_sb[0:64, i:i + 1])
            nc.scalar.activation(st["qkTo"][:, i, :], pq[64:128, :],
                                 AF.Identity, bias=bqk_sb[64:128, i:i + 1])

        def front_piece(st, ck, piece):
            """piece 0: xT DMA (latency hidden behind pairs 0-1);
            pieces 2-5: one V slice each; pieces 6-11: qk tiles 2i, 2i+1."""
            if piece == 0:
                front_xt(st, ck)
            elif 2 <= piece < 6:
                front_v_slice(st, piece - 2)
            elif 6 <= piece < 12:
                front_qk_tile(st, 2 * (piece - 6))
                front_qk_tile(st, 2 * (piece - 6) + 1)

        def front_all(st, ck):
            # prologue order: xT DMA, then qk (pairs need it immediately),
            # then V (first needed at pair PVLAG)
            for piece in [0, 6, 7, 8, 9, 10, 11, 2, 3, 4, 5]:
                front_piece(st, ck, piece)

        # ---------- back work: proj of a finished chunk ----------
        phs = {}

        def emit_proj_half(ot, ck, ti, half):
            off, rows = TOK_TILES[ti]
            ph = ps_ph.tile([128, 384], f32, tag="ph", name="ph")[:rows]
            for j in range(6):
                nc.tensor.matmul(
                    ph, ot[:, j, off:off + rows],
                    wp_sb[:, j, half * 384:(half + 1) * 384],
                    start=(j == 0), stop=(j == 5))
            if half == 0:
                phs[ti % 2] = (ph, y_p.tile([128, C], f32, tag="y", name="ysb"))
            else:
                phs[ti % 2] = (phs[ti % 2][0], phs[ti % 2][1], ph)
                ph0, y_sb, ph1 = phs[ti % 2]
                for h, p in ((0, ph0), (1, ph1)):
                    nc.vector.tensor_tensor(
                        y_sb[:rows, h * 384:(h + 1) * 384], p,
                        bp_sb[:rows, h * 384:(h + 1) * 384], OP.add)
                nc.scalar.dma_start(
                    y[ck * T + off: ck * T + off + rows, :], y_sb[:rows])

        def emit_proj(ot, ck):
            for ti in range(len(TOK_TILES)):
                for half in range(2):
                    emit_proj_half(ot, ck, ti, half)

        # ---------- attention ----------
        def emit_s(st, p):
            b, j = divmod(p, 6)
            sts = []
            for mi, (moff, mrows) in enumerate(MC):
                s_t = ps_st.tile([98, 2, N], f32, tag="st")
                for hp in range(2):
                    qs = st["qkTe"] if hp == 0 else st["qkTo"]
                    k_ap = qs[:, 6 + j, b * N + moff: b * N + moff + mrows]
                    q_ap = qs[:, j, b * N: (b + 1) * N]
                    nc.tensor.matmul(
                        s_t[:mrows, hp, :], k_ap, q_ap,
                        start=True, stop=True)
                sts.append(s_t)
            return sts

        def emit_p(st, p, sts):
            b, j = divmod(p, 6)
            pt = p_p.tile([128, 4, N], bf16, tag="p")
            for mi in range(2):
                m_sb = (mA_sb if mi == 0 else mB_sb)
                nc.vector.tensor_tensor(
                    pt[:98, 2 * mi:2 * mi + 2, :], sts[mi][:],
                    m_sb[:, 2 * j:2 * j + 2, :], OP.mult)
            nc.scalar.activation(pt[:98], pt[:98], AF.Exp)
            return pt

        def emit_pv(st, ot, pend):
            b, j, pt = pend
            po = ps_o.tile([D + 1, 2, N], f32, tag="o")
            for hp in range(2):
                for mi, (moff, mrows) in enumerate(MC):
                    nc.tensor.matmul(
                        po[:, hp, :],
                        st["vts"][b * 2 + mi][:mrows, 2 * j + hp, :],
                        pt[:mrows, 2 * mi + hp, :],
                        start=(mi == 0), stop=(mi == 1))
            # unload PSUM immediately (frees the po slot): reciprocal of the
            # denominator row on DVE (no ACT table holds exp+reciprocal, so
            # ACT reciprocal would thrash 1.3us table reloads), raw attention
            # rows to SBUF on ACT. Normalization happens per batch in
            # emit_norm_batch.
            if "rt" not in st or st["rt_b"] != b:
                st["rt"] = rc_p.tile([1, 6, 2, N], f32, tag="rc", name="rt")
                st["orb"] = or_p.tile([64, 6, 2, N], bf16, tag="or",
                                      name="orb")
                st["rt_b"] = b
            nc.vector.reciprocal(st["rt"][:, j, :, :], po[D:D + 1, :, :])
            nc.scalar.activation(st["orb"][:, j, :, :], po[:D, :, :],
                                 AF.Copy)
            return (b, st["rt"], st["orb"])

        def emit_norm_batch(ot, b, rt, orb):
            """Batched softmax normalization: one DRAM round-trip broadcasts
            1/den for all 6 head-pairs of a batch (engines cannot
            partition-broadcast), then two wide GpSimd multiplies write the
            normalized attention output (partition-base shift for odd heads
            happens on GpSimd)."""
            scr = dram_p.tile([1, 6, 2, N], f32, tag="scr", name="scr")
            nc.sync.dma_start(scr[:], rt[:])
            bcast = bc_p.tile([64, 6, 2, N], f32, tag="bc")
            scr_ap = scr[:]
            nc.sync.dma_start(
                bcast[:],
                bass.AP(tensor=scr_ap.tensor, offset=scr_ap.offset,
                        ap=[[0, 64], [2 * N, 6], [N, 2], [1, N]]))
            for hp in range(2):
                nc.gpsimd.tensor_tensor(
                    ot[hp * 64:(hp + 1) * 64, :, b * N:(b + 1) * N],
                    orb[:, :, hp, :], bcast[:, :, hp, :], OP.mult)

        loop_cm = tc.For_i(0, loop, 1) if loop else nullcontext()
        with loop_cm:
          for _rep in range(repeat):
            # chunks[ck] = {"st": front state, "ot": attention output tile}
            chunks = {0: {"st": {}, "ot": None}}
            front_all(chunks[0]["st"], 0)
            if stage == 1:
                nc.sync.dma_start(
                    y[0:128, 0:196],
                    chunks[0]["st"]["qkTe"][:, 0, :].bitcast(f32))
                continue
            pends = []   # (ck, (b, j, pt)) awaiting PV
            nchunk = 1 if stage == 2 else NCHUNK
            for ck in range(nchunk):
                rec = chunks[ck]
                rec["ot"] = ot_p.tile([128, 6, T], bf16, tag="ot", name="ott")
                if ck + 1 < nchunk:
                    chunks[ck + 1] = {"st": {}, "ot": None}
                for p in range(NPAIR):
                    sts = emit_s(rec["st"], p)
                    if len(pends) >= PVLAG:
                        cko, pend = pends.pop(0)
                        done = emit_pv(chunks[cko]["st"], chunks[cko]["ot"],
                                       pend)
                        if pend[1] == 5:
                            emit_norm_batch(chunks[cko]["ot"], *done)
                    if ck + 1 < nchunk:
                        front_piece(chunks[ck + 1]["st"], ck + 1, p)
                    if ck >= 1 and 4 <= p < 12:
                        ti, half = divmod(p - 4, 2)
                        emit_proj_half(chunks[ck - 1]["ot"], ck - 1, ti, half)
                    pt = emit_p(rec["st"], p, sts)
                    pends.append((ck, (p // 6, p % 6, pt)))
                if ck - 2 in chunks:
                    del chunks[ck - 2]
            # epilogue: flush remaining PVs, then last chunk's proj
            while pends:
                cko, pend = pends.pop(0)
                done = emit_pv(chunks[cko]["st"], chunks[cko]["ot"], pend)
                if pend[1] == 5:
                    emit_norm_batch(chunks[cko]["ot"], *done)
            if stage == 2:
                nc.sync.dma_start(
                    y[0:128, 0:196],
                    chunks[0]["ot"][:, 0, :].bitcast(f32))
                continue
            emit_proj(chunks[NCHUNK - 1]["ot"], NCHUNK - 1)

    nc.compile()
    return nc


def _get_nc(repeat=1, loop=0, stage=3):
    key = ("nc", repeat, loop, stage)
    if key not in _CACHE:
        _CACHE[key] = _build(repeat, loop, stage)
    return _CACHE[key]


def _prep_shared(W_qkv, b_qkv, att_mask, W_proj, b_proj):
    import ml_dtypes
    bf16 = ml_dtypes.bfloat16
    W_qkv = np.asarray(W_qkv, np.float32)
    W_proj = np.asarray(W_proj, np.float32)
    b_qkv = np.asarray(b_qkv, np.float32)
    b_proj = np.asarray(b_proj, np.float32)
    att_mask = np.asarray(att_mask, np.float32)
    sig = SCALE / (1.0 + np.exp(-att_mask))          # [H, n, m]
    maskT = np.ascontiguousarray(sig.transpose(0, 2, 1))  # [H, m, n]
    return {
        "wqkT": np.ascontiguousarray(W_qkv[:2 * C].T).astype(bf16),
        "wvT": np.ascontiguousarray(W_qkv[2 * C:].T).astype(bf16),
        "wpT": np.ascontiguousarray(W_proj.T).astype(bf16),
        "bqk": np.ascontiguousarray(b_qkv[:2 * C].reshape(12, 128).T),
        "bv": np.ascontiguousarray(b_qkv[2 * C:].reshape(1, C)),
        "bp": np.ascontiguousarray(b_proj.reshape(1, C)),
        "maskA": np.ascontiguousarray(
            maskT[:, :98, :].transpose(1, 0, 2)).astype(bf16),
        "maskB": np.ascontiguousarray(
            maskT[:, 98:, :].transpose(1, 0, 2)).astype(bf16),
    }


def _make_in_maps(x, W_qkv, b_qkv, att_mask, W_proj, b_proj):
    import ml_dtypes
    bf16 = ml_dtypes.bfloat16
    x = np.asarray(x, np.float32).astype(bf16)
    shared = _prep_shared(W_qkv, b_qkv, att_mask, W_proj, b_proj)
    in_maps = []
    for c in range(NCORES):
        m = dict(shared)
        m["xT"] = np.ascontiguousarray(
            x[c * BPC:(c + 1) * BPC].reshape(BPC * N, C).T)
        in_maps.append(m)
    return in_maps


def kernel(x, W_qkv, b_qkv, att_mask, W_proj, b_proj):
    from concourse.bass_utils import run_bass_kernel_spmd

    nc = _get_nc()
    in_maps = _make_in_maps(x, W_qkv, b_qkv, att_mask, W_proj, b_proj)
    res = run_bass_kernel_spmd(nc, in_maps, core_ids=list(range(NCORES)))
    out = np.stack([res.results[c]["y"].reshape(BPC, N, C)
                    for c in range(NCORES)])
    return out.reshape(B, N, C).astype(np.float32)
